# revision 17
# baseline (speedup 1.0000x reference)
"""Trainium2 Bass kernel for nn_ConditioningEncoder (cross-attention conditioning
encoder: 1x1 convs + RoPE + 4-head cross-attention + output proj + FiLM).

Sharding: data-parallel over batch. B=16 across 8 cores -> 2 batch elements per
core. No collectives.

Structure (per core, per batch element):
  - fp8e4(e4m3)+DoubleRow matmuls (K=256 in one pass, 0.5 cyc/out-col) for the
    c/k/kr/v/q/qr convs, the attention p@v, the softmax denominator Z and the
    (wo-folded) film conv.  Weights are scaled x8 on the host to clear the
    e4m3 subnormal range; the inverse scales are folded into the rope tables,
    the Z-matmul constant (OJ=4) and the film-eviction scalar (1/128) at zero
    runtime cost.
  - RoPE rotate_half folded into conv weights (wqr = R@wq) exactly; cos/sin
    combine on DVE/Pool writes bf16 q_rope/k_rope.
  - Scores S^T[s,t] = k_h^T q_h in bf16 into 2-bank PSUM groups; exp() fused
    into the PSUM->SBUF eviction on the scalar engine writing fp8 p directly
    (numerator and denominator use the SAME quantized p, so softmax still
    sums to 1).
  - Attention output head-PAIR packed: block-diagonal fp8 stationary
    [ki, 2(head), 128] -> one DoubleRow matmul series yields both heads in one
    full PSUM bank; Z via a block-constant stationary into a second bank
    (128-row replicas); ONE reciprocal + ONE multiply per pair normalizes.
  - w_film@wo, bo and bv folded on the host into one film conv; final FiLM
    (x*gamma+beta) via two scalar_tensor_tensor ops (DVE + Pool via a DMA
    PSUM->SBUF bridge, since GPSIMD has no PSUM port).

Masks are all-ones by problem spec, so the reference's where()/final multiply
are identities and are elided.
"""

import numpy as np
import ml_dtypes

HIDDEN = 256
COND = 512
TT = 2048
TS = 512
H = 4
KC = 64
N_CORES = 8
B_FULL = 16
BPC = B_FULL // N_CORES  # batch elements per core

WS = 8.0  # fp8 weight scale

_CACHE = {}


def _rot_fold(w):
    """rotate_half as a signed row permutation applied to conv weight rows."""
    wr = np.empty_like(w)
    for h in range(H):
        b = KC * h
        wr[b : b + 32] = -w[b + 32 : b + 64]
        wr[b + 32 : b + 64] = w[b : b + 32]
    return wr


def _rope_tables(T):
    """Channel-major cos/sin tables [128, T]; rows repeat with period 64 and
    within a head rows j and j+32 share a frequency."""
    inv = 1.0 / (10000.0 ** (np.arange(0, KC, 2, dtype=np.float32) / KC))  # [32]
    t = np.arange(T, dtype=np.float32)
    f = t[None, :] * inv[:, None]  # [32, T]
    f64 = np.concatenate([f, f], 0)  # [64, T]
    f128 = np.concatenate([f64, f64], 0)  # [128, T]
    return np.cos(f128).astype(np.float32), np.sin(f128).astype(np.float32)


def _chunkT(w, n, p=128):
    """W [O, I] -> W.T chunked: [p, n, O] with [ki, k, :] = W[:, p*k + ki].T"""
    return np.ascontiguousarray(w.T.reshape(n, p, w.shape[0]).transpose(1, 0, 2))


def _colchunks(b, n, p=128):
    """bias [n*p] -> [p, n] with column m = chunk m."""
    return np.ascontiguousarray(b.reshape(n, p).T)


def _f8(a):
    return np.ascontiguousarray(
        np.asarray(a, np.float32).astype(ml_dtypes.float8_e4m3fn).view(np.uint8))


def _b16(a):
    return np.ascontiguousarray(
        np.asarray(a, np.float32).astype(ml_dtypes.bfloat16).view(np.uint16))


def _build_program(has_bias):
    from concourse import bacc, mybir, tile

    dt = mybir.dt
    f32 = dt.float32
    f8 = dt.float8e4
    bf = dt.bfloat16
    u8 = dt.uint8
    u16 = dt.uint16
    Alu = mybir.AluOpType
    Act = mybir.ActivationFunctionType
    DR = mybir.MatmulPerfMode.DoubleRow

    nc = bacc.Bacc(
        "TRN2",
        target_bir_lowering=False,
        debug=False,
        enable_asserts=False,
        num_devices=N_CORES,
    )

    d_x = nc.dram_tensor("x", [BPC, HIDDEN, TT], f32, kind="ExternalInput")
    d_cond = nc.dram_tensor("cond", [BPC, COND, TS], f32, kind="ExternalInput")
    d_cosq = nc.dram_tensor("cosq", [64, TT], u16, kind="ExternalInput")
    d_sinq = nc.dram_tensor("sinq", [64, TT], u16, kind="ExternalInput")
    d_cosk = nc.dram_tensor("cosk", [64, TS], u16, kind="ExternalInput")
    d_sink = nc.dram_tensor("sink", [64, TS], u16, kind="ExternalInput")
    d_wcT = nc.dram_tensor("wcT", [128, 4, 256], u8, kind="ExternalInput")
    d_wqT = nc.dram_tensor("wqT", [128, 2, 256], u8, kind="ExternalInput")
    d_wqrT = nc.dram_tensor("wqrT", [128, 2, 256], u8, kind="ExternalInput")
    d_wkT = nc.dram_tensor("wkT", [128, 2, 256], u8, kind="ExternalInput")
    d_wkrT = nc.dram_tensor("wkrT", [128, 2, 256], u8, kind="ExternalInput")
    d_wvT = nc.dram_tensor("wvT", [128, 2, 256], u8, kind="ExternalInput")
    d_wfoT = nc.dram_tensor("wfoT", [128, 2, 512], u8, kind="ExternalInput")
    d_oj = nc.dram_tensor("oj", [128, 2, 128], u8, kind="ExternalInput")
    d_bias = nc.dram_tensor("bias", [128, 14], f32, kind="ExternalInput")
    d_out = nc.dram_tensor("out", [BPC, HIDDEN, TT], f32, kind="ExternalOutput")

    with tile.TileContext(nc) as tc:
        with (
            tc.tile_pool(name="wp", bufs=1) as wp,
            tc.tile_pool(name="mp", bufs=2) as mp,
            tc.tile_pool(name="pp", bufs=2, space="PSUM") as pp,
        ):
            # ---- persistent tables / weights ----
            cosq = wp.tile([128, TT], bf)
            sinq = wp.tile([128, TT], bf)
            cosk = wp.tile([128, TS], bf)
            sink = wp.tile([128, TS], bf)
            wcT = wp.tile([128, 4, 256], f8)
            wqT = wp.tile([128, 2, 256], f8)
            wqrT = wp.tile([128, 2, 256], f8)
            wkT = wp.tile([128, 2, 256], f8)
            wkrT = wp.tile([128, 2, 256], f8)
            wvT = wp.tile([128, 2, 256], f8)
            wfoT = wp.tile([128, 2, 512], f8)
            oj = wp.tile([128, 2, 128], f8)
            bias = wp.tile([128, 14], f32)
            def load_weights_early():
                # everything the cond-side convs need, loaded first.  The rope
                # tables' rows repeat with period 64, so only 64 rows are
                # DMA'd and the top half is mirrored on-chip.
                for t, d in [
                    (bias, d_bias), (wcT, d_wcT), (wkT, d_wkT), (wkrT, d_wkrT),
                    (cosk, d_cosk), (sink, d_sink), (wvT, d_wvT), (oj, d_oj),
                ]:
                    nc.sync.dma_start(t[0:64] if t in (cosk, sink) else t[:],
                                      d[:].bitcast(t.dtype))
                for t in (cosk, sink):
                    nc.vector.tensor_copy(t[64:128, :], t[0:64, :])

            def load_weights_late():
                for t, d in [
                    (wqT, d_wqT), (wqrT, d_wqrT), (cosq, d_cosq), (sinq, d_sinq),
                    (wfoT, d_wfoT),
                ]:
                    nc.sync.dma_start(t[0:64] if t in (cosq, sinq) else t[:],
                                      d[:].bitcast(t.dtype))
                for t in (cosq, sinq):
                    nc.vector.tensor_copy(t[64:128, :], t[0:64, :])
            # bias columns: 0,1 bc | 2,3 bq | 4,5 bqr | 6,7 bk | 8,9 bkr
            #               10,11 b2gamma | 12,13 b2beta
            bc_ = lambda m: bias[:, 0 + m : 1 + m]
            bq_ = lambda m: bias[:, 2 + m : 3 + m]
            bqr_ = lambda m: bias[:, 4 + m : 5 + m]
            bk_ = lambda m: bias[:, 6 + m : 7 + m]
            bkr_ = lambda m: bias[:, 8 + m : 9 + m]
            bfg_ = lambda m: bias[:, 10 + m : 11 + m]
            bfb_ = lambda m: bias[:, 12 + m : 13 + m]

            st = [dict() for _ in range(BPC)]  # per-batch tile state

            def load_cond(b, fast):
                s = st[b]
                c32 = mp.tile([128, 4, TS], f32, tag="c32", bufs=2, name=f"c32{b}")
                for kk in range(4):
                    nc.sync.dma_start(
                        c32[:, kk, :], d_cond[b, kk * 128 : kk * 128 + 128, :]
                    )
                cf = mp.tile([128, 4, TS], f8, tag="condf8", bufs=2, name=f"condf8{b}")
                if fast:
                    # first K-pair on DVE so conv_c starts early
                    nc.vector.tensor_copy(cf[:, 0:2, :], c32[:, 0:2, :])
                    nc.gpsimd.tensor_copy(cf[:, 2:4, :], c32[:, 2:4, :])
                else:
                    nc.gpsimd.tensor_copy(cf[:], c32[:])
                s["condf8"] = cf

            def load_x(b, fast):
                s = st[b]
                s["x32"] = []
                for ch in range(2):
                    xt = mp.tile([128, TT], f32, tag="x", bufs=4, name=f"x{b}{ch}")
                    if fast:
                        nc.sync.dma_start(
                            xt[:, 0:1024], d_x[b, ch * 128 : ch * 128 + 128, 0:1024]
                        )
                    else:
                        nc.sync.dma_start(xt[:], d_x[b, ch * 128 : ch * 128 + 128, :])
                    s["x32"].append(xt)
                if fast:
                    for ch in range(2):
                        nc.sync.dma_start(
                            s["x32"][ch][:, 1024:2048],
                            d_x[b, ch * 128 : ch * 128 + 128, 1024:2048],
                        )
                xf8 = mp.tile([128, 2, TT], f8, tag="xf8", bufs=2, name=f"xf8{b}")
                if fast:
                    for ch in range(2):
                        nc.vector.tensor_copy(
                            xf8[:, ch, 0:1024], s["x32"][ch][:, 0:1024]
                        )
                    for ch in range(2):
                        nc.gpsimd.tensor_copy(
                            xf8[:, ch, 1024:2048], s["x32"][ch][:, 1024:2048]
                        )
                else:
                    for ch in range(2):
                        nc.gpsimd.tensor_copy(xf8[:, ch, :], s["x32"][ch][:])
                s["xf8"] = xf8

            def conv_c(b):
                s = st[b]
                cf8 = mp.tile([128, 2, TS], f8, tag="c", bufs=2, name=f"c{b}")
                for m in range(2):
                    ps = pp.tile([128, 512], f32, tag="cv", bufs=2, name=f"psc{b}{m}")
                    for kp in range(2):
                        nc.tensor.matmul(
                            ps[:],
                            wcT[:, 2 * kp : 2 * kp + 2, m * 128 : m * 128 + 128],
                            s["condf8"][:, 2 * kp : 2 * kp + 2, :],
                            start=(kp == 0), stop=(kp == 1), perf_mode=DR,
                        )
                    # cf8 = WS*c + WS*bc  (bias column pre-scaled by WS on host)
                    nc.vector.tensor_scalar_add(cf8[:, m, :], ps[:], bc_(m))
                s["cf8"] = cf8

            def conv_k(b):
                s = st[b]
                s["krope"] = []
                for m in range(2):
                    psk = pp.tile([128, 512], f32, tag="cv", bufs=2, name=f"psk{b}{m}")
                    pskr = pp.tile([128, 512], f32, tag="cv", bufs=2, name=f"pskr{b}{m}")
                    nc.tensor.matmul(
                        psk[:], wkT[:, :, m * 128 : m * 128 + 128], s["cf8"][:],
                        start=True, stop=True, perf_mode=DR,
                    )
                    nc.tensor.matmul(
                        pskr[:], wkrT[:, :, m * 128 : m * 128 + 128], s["cf8"][:],
                        start=True, stop=True, perf_mode=DR,
                    )
                    t1 = mp.tile([128, TS], bf, tag="kt1", bufs=2, name=f"kt1{b}{m}")
                    t2 = mp.tile([128, TS], bf, tag="kt2", bufs=2, name=f"kt2{b}{m}")
                    # (64*k + 64*bk) * (cos/64) == k_rope exactly
                    nc.vector.scalar_tensor_tensor(
                        t1[:], psk[:], bk_(m), cosk[:], op0=Alu.add, op1=Alu.mult
                    )
                    nc.vector.scalar_tensor_tensor(
                        t2[:], pskr[:], bkr_(m), sink[:], op0=Alu.add, op1=Alu.mult
                    )
                    kr = mp.tile([128, TS], bf, tag="krope", bufs=2, name=f"krope{b}{m}")
                    nc.gpsimd.tensor_add(kr[:], t1[:], t2[:])
                    s["krope"].append(kr)

            def conv_vt(b):
                s = st[b]
                # block-diagonal v^T: [s128, sc4, hp2, j2, 128]
                vt = mp.tile([128, 4, 2, 2, 128], f8, tag="vt", bufs=2, name=f"vt{b}")
                nc.gpsimd.memset(vt[:, :, :, 0, 64:128], 0.0)
                nc.gpsimd.memset(vt[:, :, :, 1, 0:64], 0.0)
                for sc in range(4):
                    ps = pp.tile([128, 4, 128], f32, tag="cv", bufs=2, name=f"psvt{b}{sc}")
                    nc.tensor.matmul(
                        ps[:, 0:2, :],
                        s["cf8"][:, :, sc * 128 : sc * 128 + 128], wvT[:],
                        start=True, stop=True, perf_mode=DR,
                    )
                    # j=0 rows: heads 0,2 (cols 0:64 of each pair)
                    nc.vector.tensor_copy(vt[:, sc, :, 0, 0:64], ps[:, 0:2, 0:64])
                    nc.vector.tensor_copy(vt[:, sc, :, 1, 64:128], ps[:, 0:2, 64:128])
                s["vt"] = vt

            def conv_q(b, nb):
                """q/qr conv + rope for one 512-wide t-chunk (feeds attn(b, nb))."""
                s = st[b]
                if nb == 0:
                    s["qrope"] = [
                        mp.tile([128, 4, 512], bf, tag="qrope", bufs=4, name=f"qrope{b}{m}")
                        for m in range(2)
                    ]
                sl = slice(nb * 512, nb * 512 + 512)
                for m in range(2):
                    psq = pp.tile([128, 512], f32, tag="cv", bufs=2, name=f"psq{b}{m}{nb}")
                    psqr = pp.tile([128, 512], f32, tag="cv", bufs=2, name=f"psqr{b}{m}{nb}")
                    nc.tensor.matmul(
                        psq[:], wqT[:, :, m * 128 : m * 128 + 128],
                        s["xf8"][:, :, nb * 512 : nb * 512 + 512],
                        start=True, stop=True, perf_mode=DR,
                    )
                    nc.tensor.matmul(
                        psqr[:], wqrT[:, :, m * 128 : m * 128 + 128],
                        s["xf8"][:, :, nb * 512 : nb * 512 + 512],
                        start=True, stop=True, perf_mode=DR,
                    )
                    t1 = mp.tile([128, 512], bf, tag="qt1", bufs=2, name=f"qt1{b}{m}{nb}")
                    nc.vector.scalar_tensor_tensor(
                        t1[:], psq[:], bq_(m), cosq[:, sl], op0=Alu.add, op1=Alu.mult
                    )
                    t2 = mp.tile([128, 512], bf, tag="qt2", bufs=2, name=f"qt2{b}{m}{nb}")
                    nc.vector.scalar_tensor_tensor(
                        t2[:], psqr[:], bqr_(m), sinq[:, sl], op0=Alu.add, op1=Alu.mult
                    )
                    nc.gpsimd.tensor_add(s["qrope"][m][:, nb, :], t1[:], t2[:])

            def attn(b, tq):
                s = st[b]
                if tq == 0:
                    s["ntp"] = {}
                ntp = mp.tile([128, 2, 512], f8, tag="ntp", bufs=3, name=f"ntp{b}{tq}")
                s["ntp"][tq] = ntp
                ps_p = {}
                for hp in range(2):
                    # p tile [s128, head-in-pair 2, sc 4, t 512]
                    p = mp.tile([128, 2, 4, 512], f8, tag="p", bufs=3, name=f"p{b}{tq}{hp}")
                    ps_p[hp] = p
                for h in range(H):
                    hp, hh = divmod(h, 2)
                    chq = h // 2
                    base = 64 * (h % 2)
                    p = ps_p[hp]
                    for e in range(2):
                        pss = pp.tile([128, 2, 512], f32, tag="pss", bufs=2,
                                      name=f"pss{b}{tq}{h}{e}")
                        for j in range(2):
                            sc = 2 * e + j
                            nc.tensor.matmul(
                                pss[:, j, :],
                                s["krope"][chq][base : base + 64, sc * 128 : sc * 128 + 128],
                                s["qrope"][chq][base : base + 64, tq, :],
                                start=True, stop=True,
                            )
                        nc.scalar.activation(
                            p[:, hh, 2 * e : 2 * e + 2, :], pss[:], Act.Exp, scale=0.125
                        )
                    if h % 2 == 1:
                        # pair complete: attention out + Z + normalize
                        p = ps_p[hp]
                        pso = pp.tile([128, 512], f32, tag="att", bufs=2,
                                      name=f"pso{b}{tq}{hp}")
                        for sc in range(4):
                            nc.tensor.matmul(
                                pso[:], s["vt"][:, sc, hp, :, :], p[:, :, sc, :],
                                start=(sc == 0), stop=(sc == 3), perf_mode=DR,
                            )
                        zb = pp.tile([128, 512], f32, tag="att", bufs=2,
                                     name=f"zb{b}{tq}{hp}")
                        for sc in range(4):
                            nc.tensor.matmul(
                                zb[:], oj[:], p[:, :, sc, :],
                                start=(sc == 0), stop=(sc == 3), perf_mode=DR,
                            )
                        zr = mp.tile([128, 512], f32, tag="zr", bufs=2,
                                     name=f"zr{b}{tq}{hp}")
                        nc.vector.reciprocal(zr[:], zb[:])
                        # ntp = pso * zr = (64*a_hat) / (4*Z) = 16*a
                        nc.vector.tensor_mul(ntp[:, hp, :], pso[:], zr[:])

            def film(b, tq, beta_on_act):
                s = st[b]
                if tq == 0:
                    s["ft"] = []
                    for ch in range(2):
                        ft = mp.tile([128, TT], f32, tag="ft", bufs=4, name=f"ft{b}{ch}")
                        s["ft"].append(ft)
                ntp = s["ntp"][tq]
                tsl = slice(tq * 512, tq * 512 + 512)
                for ch in range(2):
                    psf = pp.tile([128, 2, 512], f32, tag="pss", bufs=2,
                                  name=f"psf{b}{tq}{ch}")
                    nc.tensor.matmul(
                        psf[:, 0, :], wfoT[:, :, ch * 128 : ch * 128 + 128], ntp[:],
                        start=True, stop=True, perf_mode=DR,
                    )
                    nc.tensor.matmul(
                        psf[:, 1, :], wfoT[:, :, 256 + ch * 128 : 256 + ch * 128 + 128],
                        ntp[:],
                        start=True, stop=True, perf_mode=DR,
                    )
                    # tg = (psf_gamma/128) * x ; psf = (8wfo)@(16a) = 128*gamma
                    tg = mp.tile([128, 512], f32, tag="tg", bufs=2, name=f"tg{b}{tq}{ch}")
                    nc.vector.scalar_tensor_tensor(
                        tg[:], psf[:, 0, :], 1.0 / 128.0, s["x32"][ch][:, tsl],
                        op0=Alu.mult, op1=Alu.mult,
                    )
                    ftsl = s["ft"][ch][:, tsl]
                    if beta_on_act:
                        # evict beta on the scalar engine, add on Pool (both
                        # off the DVE critical path)
                        fb = mp.tile([128, 512], f32, tag="fb", bufs=2,
                                     name=f"fb{b}{tq}{ch}")
                        if has_bias:
                            nc.scalar.activation(
                                fb[:], psf[:, 1, :], Act.Identity,
                                bias=bfb_(ch), scale=1.0 / 128.0,
                            )
                        else:
                            nc.scalar.activation(
                                fb[:], psf[:, 1, :], Act.Copy, scale=1.0 / 128.0,
                            )
                        nc.gpsimd.tensor_add(ftsl, fb[:], tg[:])
                    else:
                        nc.vector.scalar_tensor_tensor(
                            ftsl, psf[:, 1, :], 1.0 / 128.0, tg[:],
                            op0=Alu.mult, op1=Alu.add,
                        )
                        if has_bias:
                            nc.vector.tensor_scalar_add(ftsl, ftsl, bfb_(ch))
                    if has_bias:
                        # y += x*b2gamma (skipped when biases are zero)
                        nc.vector.scalar_tensor_tensor(
                            ftsl, s["x32"][ch][:, tsl], bfg_(ch), ftsl,
                            op0=Alu.mult, op1=Alu.add,
                        )
                    nc.sync.dma_start(
                        d_out[b, ch * 128 : ch * 128 + 128, tsl], ftsl
                    )

            # ---- emission schedule: per-t-chunk pipeline, overlap batch-1
            # convs with batch-0 attention ----
            load_weights_early()
            load_cond(0, fast=True)
            load_x(0, fast=True)
            load_weights_late()
            conv_c(0); conv_k(0); conv_vt(0)
            conv_q(0, 0); conv_q(0, 1)
            attn(0, 0)
            load_cond(1, fast=False); load_x(1, fast=False)
            conv_q(0, 2); attn(0, 1); film(0, 0, True)
            conv_q(0, 3); attn(0, 2); film(0, 1, True); conv_c(1)
            conv_k(1); conv_vt(1)
            conv_q(1, 0); attn(0, 3); film(0, 2, True)
            conv_q(1, 1); attn(1, 0); film(0, 3, True)
            conv_q(1, 2); attn(1, 1); film(1, 0, False)
            conv_q(1, 3); attn(1, 2); film(1, 1, False)
            attn(1, 3); film(1, 2, False)
            film(1, 3, False)

    nc.compile()
    return nc


def _host_prep(inputs):
    wq, bq = inputs["wq"], inputs["bq"]
    wk, bk = inputs["wk"], inputs["bk"]
    wv, bv = inputs["wv"], inputs["bv"]
    wc, bc = inputs["w_cond"], inputs["b_cond"]
    wo = inputs["wo"]
    wf, bf_ = inputs["w_film"], inputs["b_film"]

    cosq, sinq = _rope_tables(TT)
    cosk, sink = _rope_tables(TS)
    # fold output projection and bv/bo into the film conv (host fp64)
    wfo = (wf.astype(np.float64) @ wo.astype(np.float64)).astype(np.float32)
    b2 = (
        wfo.astype(np.float64) @ bv.astype(np.float64)
        + wf.astype(np.float64) @ inputs["bo"].astype(np.float64)
        + bf_
    ).astype(np.float32)

    ojv = np.zeros((128, 2, 128), np.float32)
    ojv[:, 0, 0:64] = 4.0
    ojv[:, 1, 64:128] = 4.0

    bias = np.zeros((128, 14), np.float32)
    bias[:, 0:2] = _colchunks(bc, 2) * WS
    bias[:, 2:4] = _colchunks(bq, 2) * WS
    bias[:, 4:6] = _colchunks(_rot_fold(bq[:, None])[:, 0], 2) * WS
    bias[:, 6:8] = _colchunks(bk, 2) * WS * WS
    bias[:, 8:10] = _colchunks(_rot_fold(bk[:, None])[:, 0], 2) * WS * WS
    bias[:, 10:12] = _colchunks(b2[:HIDDEN], 2)
    bias[:, 12:14] = _colchunks(b2[HIDDEN:], 2)

    shared = {
        "cosq": _b16(cosq / WS), "sinq": _b16(sinq / WS),
        "cosk": _b16(cosk / (WS * WS)), "sink": _b16(sink / (WS * WS)),
        "wcT": _f8(_chunkT(wc, 4) * WS),
        "wqT": _f8(_chunkT(wq, 2) * WS),
        "wqrT": _f8(_chunkT(_rot_fold(wq), 2) * WS),
        "wkT": _f8(_chunkT(wk, 2) * WS),
        "wkrT": _f8(_chunkT(_rot_fold(wk), 2) * WS),
        "wvT": _f8(_chunkT(wv, 2) * WS),
        "wfoT": _f8(_chunkT(wfo, 2) * WS),
        "oj": _f8(ojv),
        "bias": np.ascontiguousarray(bias),
    }
    has_bias = bool(np.any(b2 != 0.0))
    return shared, has_bias


def kernel(**inputs):
    from concourse.bass_utils import run_bass_kernel_spmd

    inputs = {k: np.asarray(v, dtype=np.float32) for k, v in inputs.items()}
    shared, has_bias = _host_prep(inputs)

    key = ("nc", has_bias)
    if key not in _CACHE:
        _CACHE["nc"] = _build_program(has_bias)
        _CACHE[key] = _CACHE["nc"]
    nc = _CACHE[key]

    x = inputs["x"]
    cond = inputs["cond_latent"]
    in_maps = []
    for c in range(N_CORES):
        m = dict(shared)
        m["x"] = np.ascontiguousarray(x[c * BPC : (c + 1) * BPC])
        m["cond"] = np.ascontiguousarray(cond[c * BPC : (c + 1) * BPC])
        in_maps.append(m)

    res = run_bass_kernel_spmd(nc, in_maps, list(range(N_CORES)))
    out = np.concatenate([res.results[c]["out"] for c in range(N_CORES)], axis=0)
    return out.astype(np.float32)


# revision 24
# speedup vs baseline: 1.0359x; 1.0359x over previous
"""Trainium2 Bass kernel for nn_ConditioningEncoder (cross-attention conditioning
encoder: 1x1 convs + RoPE + 4-head cross-attention + output proj + FiLM).

Sharding: data-parallel over batch. B=16 across 8 cores -> 2 batch elements per
core. No collectives.

Structure (per core, per batch element):
  - fp8e4(e4m3)+DoubleRow matmuls (K=256 in one pass, 0.5 cyc/out-col) for the
    c/k/kr/v/q/qr convs, the attention p@v, the softmax denominator Z and the
    (wo-folded) film conv.  Weights are scaled x8 on the host to clear the
    e4m3 subnormal range; the inverse scales are folded into the rope tables,
    the Z-matmul constant (OJ=4) and the film-eviction scalar (1/128) at zero
    runtime cost.
  - RoPE rotate_half folded into conv weights (wqr = R@wq) exactly; cos/sin
    combine on DVE/Pool writes bf16 q_rope/k_rope.
  - Scores S^T[s,t] = k_h^T q_h in bf16 into 2-bank PSUM groups; exp() fused
    into the PSUM->SBUF eviction on the scalar engine writing fp8 p directly
    (numerator and denominator use the SAME quantized p, so softmax still
    sums to 1).
  - Attention output head-PAIR packed: block-diagonal fp8 stationary
    [ki, 2(head), 128] -> one DoubleRow matmul series yields both heads in one
    full PSUM bank; Z via a block-constant stationary into a second bank
    (128-row replicas); ONE reciprocal + ONE multiply per pair normalizes.
  - w_film@wo, bo and bv folded on the host into one film conv; final FiLM
    (x*gamma+beta) via two scalar_tensor_tensor ops (DVE + Pool via a DMA
    PSUM->SBUF bridge, since GPSIMD has no PSUM port).

Masks are all-ones by problem spec, so the reference's where()/final multiply
are identities and are elided.
"""

import numpy as np
import ml_dtypes

HIDDEN = 256
COND = 512
TT = 2048
TS = 512
H = 4
KC = 64
N_CORES = 8
B_FULL = 16
BPC = B_FULL // N_CORES  # batch elements per core

WS = 8.0  # fp8 weight scale

_CACHE = {}


def _rot_fold(w):
    """rotate_half as a signed row permutation applied to conv weight rows."""
    wr = np.empty_like(w)
    for h in range(H):
        b = KC * h
        wr[b : b + 32] = -w[b + 32 : b + 64]
        wr[b + 32 : b + 64] = w[b : b + 32]
    return wr


def _rope_tables(T):
    """Channel-major cos/sin tables [128, T]; rows repeat with period 64 and
    within a head rows j and j+32 share a frequency."""
    inv = 1.0 / (10000.0 ** (np.arange(0, KC, 2, dtype=np.float32) / KC))  # [32]
    t = np.arange(T, dtype=np.float32)
    f = t[None, :] * inv[:, None]  # [32, T]
    f64 = np.concatenate([f, f], 0)  # [64, T]
    f128 = np.concatenate([f64, f64], 0)  # [128, T]
    return np.cos(f128).astype(np.float32), np.sin(f128).astype(np.float32)


def _chunkT(w, n, p=128):
    """W [O, I] -> W.T chunked: [p, n, O] with [ki, k, :] = W[:, p*k + ki].T"""
    return np.ascontiguousarray(w.T.reshape(n, p, w.shape[0]).transpose(1, 0, 2))


def _colchunks(b, n, p=128):
    """bias [n*p] -> [p, n] with column m = chunk m."""
    return np.ascontiguousarray(b.reshape(n, p).T)


def _f8(a):
    return np.ascontiguousarray(
        np.asarray(a, np.float32).astype(ml_dtypes.float8_e4m3fn).view(np.uint8))


def _b16(a):
    return np.ascontiguousarray(
        np.asarray(a, np.float32).astype(ml_dtypes.bfloat16).view(np.uint16))


def _build_program(has_bias):
    from concourse import bacc, mybir, tile

    dt = mybir.dt
    f32 = dt.float32
    f8 = dt.float8e4
    bf = dt.bfloat16
    u8 = dt.uint8
    u16 = dt.uint16
    Alu = mybir.AluOpType
    Act = mybir.ActivationFunctionType
    DR = mybir.MatmulPerfMode.DoubleRow

    nc = bacc.Bacc(
        "TRN2",
        target_bir_lowering=False,
        debug=False,
        enable_asserts=False,
        num_devices=N_CORES,
    )

    d_x = nc.dram_tensor("x", [BPC, HIDDEN, TT], f32, kind="ExternalInput")
    d_cond = nc.dram_tensor("cond", [BPC, COND, TS], f32, kind="ExternalInput")
    d_cosq = nc.dram_tensor("cosq", [64, TT], u16, kind="ExternalInput")
    d_sinq = nc.dram_tensor("sinq", [64, TT], u16, kind="ExternalInput")
    d_cosk = nc.dram_tensor("cosk", [64, TS], u16, kind="ExternalInput")
    d_sink = nc.dram_tensor("sink", [64, TS], u16, kind="ExternalInput")
    d_wcT = nc.dram_tensor("wcT", [128, 4, 256], u8, kind="ExternalInput")
    d_wqT = nc.dram_tensor("wqT", [128, 2, 256], u8, kind="ExternalInput")
    d_wqrT = nc.dram_tensor("wqrT", [128, 2, 256], u8, kind="ExternalInput")
    d_wkT = nc.dram_tensor("wkT", [128, 2, 256], u8, kind="ExternalInput")
    d_wkrT = nc.dram_tensor("wkrT", [128, 2, 256], u8, kind="ExternalInput")
    d_wvT = nc.dram_tensor("wvT", [128, 2, 256], u8, kind="ExternalInput")
    d_wfoT = nc.dram_tensor("wfoT", [128, 2, 512], u8, kind="ExternalInput")
    d_oj = nc.dram_tensor("oj", [128, 2, 128], u8, kind="ExternalInput")
    d_bias = nc.dram_tensor("bias", [128, 14], f32, kind="ExternalInput")
    d_out = nc.dram_tensor("out", [BPC, HIDDEN, TT], f32, kind="ExternalOutput")

    with tile.TileContext(nc) as tc:
        with (
            tc.tile_pool(name="wp", bufs=1) as wp,
            tc.tile_pool(name="mp", bufs=2) as mp,
            tc.tile_pool(name="pp", bufs=2, space="PSUM") as pp,
        ):
            # ---- persistent tables / weights ----
            cosq = wp.tile([128, TT], bf)
            sinq = wp.tile([128, TT], bf)
            cosk = wp.tile([128, TS], bf)
            sink = wp.tile([128, TS], bf)
            wcT = wp.tile([128, 4, 256], f8)
            wqT = wp.tile([128, 2, 256], f8)
            wqrT = wp.tile([128, 2, 256], f8)
            wkT = wp.tile([128, 2, 256], f8)
            wkrT = wp.tile([128, 2, 256], f8)
            wvT = wp.tile([128, 2, 256], f8)
            wfoT = wp.tile([128, 2, 512], f8)
            oj = wp.tile([128, 2, 128], f8)
            bias = wp.tile([128, 14], f32)
            def load_weights_early():
                # everything the cond-side convs need, loaded first.  The rope
                # tables' rows repeat with period 64, so only 64 rows are
                # DMA'd and the top half is mirrored on-chip.
                for t, d in [
                    (bias, d_bias), (wcT, d_wcT), (wkT, d_wkT), (wkrT, d_wkrT),
                    (cosk, d_cosk), (sink, d_sink), (wvT, d_wvT), (oj, d_oj),
                ]:
                    nc.sync.dma_start(t[0:64] if t in (cosk, sink) else t[:],
                                      d[:].bitcast(t.dtype))
                for t in (cosk, sink):
                    nc.vector.tensor_copy(t[64:128, :], t[0:64, :])

            def load_tables_q():
                for t, d in [(cosq, d_cosq), (sinq, d_sinq)]:
                    nc.sync.dma_start(t[0:64], d[:].bitcast(t.dtype))
                for t in (cosq, sinq):
                    nc.vector.tensor_copy(t[64:128, :], t[0:64, :])

            def load_weights_late():
                for t, d in [(wqT, d_wqT), (wqrT, d_wqrT)]:
                    nc.sync.dma_start(t[:], d[:].bitcast(t.dtype))

            def load_wfo():
                nc.sync.dma_start(wfoT[:], d_wfoT[:].bitcast(f8))
            # bias columns: 0,1 bc | 2,3 bq | 4,5 bqr | 6,7 bk | 8,9 bkr
            #               10,11 b2gamma | 12,13 b2beta
            bc_ = lambda m: bias[:, 0 + m : 1 + m]
            bq_ = lambda m: bias[:, 2 + m : 3 + m]
            bqr_ = lambda m: bias[:, 4 + m : 5 + m]
            bk_ = lambda m: bias[:, 6 + m : 7 + m]
            bkr_ = lambda m: bias[:, 8 + m : 9 + m]
            bfg_ = lambda m: bias[:, 10 + m : 11 + m]
            bfb_ = lambda m: bias[:, 12 + m : 13 + m]

            st = [dict() for _ in range(BPC)]  # per-batch tile state

            def load_cond_dma(b):
                s = st[b]
                c32 = mp.tile([128, 4, TS], f32, tag="c32", bufs=2, name=f"c32{b}")
                for kk in range(4):
                    nc.sync.dma_start(
                        c32[:, kk, :], d_cond[b, kk * 128 : kk * 128 + 128, :]
                    )
                s["c32"] = c32

            def cast_cond(b, split):
                s = st[b]
                cf = mp.tile([128, 4, TS], f8, tag="condf8", bufs=2, name=f"condf8{b}")
                if split:
                    # first K-pair on DVE so conv_c starts early
                    nc.vector.tensor_copy(cf[:, 0:2, :], s["c32"][:, 0:2, :])
                    nc.gpsimd.tensor_copy(cf[:, 2:4, :], s["c32"][:, 2:4, :])
                else:
                    nc.gpsimd.tensor_copy(cf[:], s["c32"][:])
                s["condf8"] = cf

            def load_x_dma(b, half):
                s = st[b]
                if half == 0:
                    s["x32"] = []
                    s["xf8"] = mp.tile([128, 2, TT], f8, tag="xf8", bufs=2,
                                       name=f"xf8{b}")
                    for ch in range(2):
                        xt = mp.tile([128, TT], f32, tag="x", bufs=4, name=f"x{b}{ch}")
                        s["x32"].append(xt)
                sl = slice(half * 1024, half * 1024 + 1024)
                for ch in range(2):
                    nc.sync.dma_start(
                        s["x32"][ch][:, sl], d_x[b, ch * 128 : ch * 128 + 128, sl]
                    )

            def cast_x(b, half, eng):
                s = st[b]
                sl = slice(half * 1024, half * 1024 + 1024)
                for ch in range(2):
                    eng.tensor_copy(s["xf8"][:, ch, sl], s["x32"][ch][:, sl])

            def conv_c(b):
                s = st[b]
                cf8 = mp.tile([128, 2, TS], f8, tag="c", bufs=2, name=f"c{b}")
                for m in range(2):
                    ps = pp.tile([128, 512], f32, tag="cv", bufs=2, name=f"psc{b}{m}")
                    for kp in range(2):
                        nc.tensor.matmul(
                            ps[:],
                            wcT[:, 2 * kp : 2 * kp + 2, m * 128 : m * 128 + 128],
                            s["condf8"][:, 2 * kp : 2 * kp + 2, :],
                            start=(kp == 0), stop=(kp == 1), perf_mode=DR,
                        )
                    # cf8 = WS*c + WS*bc  (bias column pre-scaled by WS on host)
                    nc.vector.tensor_scalar_add(cf8[:, m, :], ps[:], bc_(m))
                s["cf8"] = cf8

            def conv_k(b, add_eng):
                s = st[b]
                s["krope"] = []
                for m in range(2):
                    psk = pp.tile([128, 512], f32, tag="cv", bufs=2, name=f"psk{b}{m}")
                    pskr = pp.tile([128, 512], f32, tag="cv", bufs=2, name=f"pskr{b}{m}")
                    nc.tensor.matmul(
                        psk[:], wkT[:, :, m * 128 : m * 128 + 128], s["cf8"][:],
                        start=True, stop=True, perf_mode=DR,
                    )
                    nc.tensor.matmul(
                        pskr[:], wkrT[:, :, m * 128 : m * 128 + 128], s["cf8"][:],
                        start=True, stop=True, perf_mode=DR,
                    )
                    t1 = mp.tile([128, TS], bf, tag="kt1", bufs=2, name=f"kt1{b}{m}")
                    t2 = mp.tile([128, TS], bf, tag="kt2", bufs=2, name=f"kt2{b}{m}")
                    # (64*k + 64*bk) * (cos/64) == k_rope exactly
                    nc.vector.scalar_tensor_tensor(
                        t1[:], psk[:], bk_(m), cosk[:], op0=Alu.add, op1=Alu.mult
                    )
                    nc.vector.scalar_tensor_tensor(
                        t2[:], pskr[:], bkr_(m), sink[:], op0=Alu.add, op1=Alu.mult
                    )
                    kr = mp.tile([128, TS], bf, tag="krope", bufs=2, name=f"krope{b}{m}")
                    add_eng.tensor_add(kr[:], t1[:], t2[:])
                    s["krope"].append(kr)

            def conv_vt(b):
                s = st[b]
                # block-diagonal v^T: [s128, sc4, hp2, j2, 128]
                vt = mp.tile([128, 4, 2, 2, 128], f8, tag="vt", bufs=2, name=f"vt{b}")
                nc.gpsimd.memset(vt[:, :, :, 0, 64:128], 0.0)
                nc.gpsimd.memset(vt[:, :, :, 1, 0:64], 0.0)
                for sc in range(4):
                    ps = pp.tile([128, 4, 128], f32, tag="cv", bufs=2, name=f"psvt{b}{sc}")
                    nc.tensor.matmul(
                        ps[:, 0:2, :],
                        s["cf8"][:, :, sc * 128 : sc * 128 + 128], wvT[:],
                        start=True, stop=True, perf_mode=DR,
                    )
                    # j=0 rows: heads 0,2 (cols 0:64 of each pair)
                    nc.vector.tensor_copy(vt[:, sc, :, 0, 0:64], ps[:, 0:2, 0:64])
                    nc.vector.tensor_copy(vt[:, sc, :, 1, 64:128], ps[:, 0:2, 64:128])
                s["vt"] = vt

            def conv_q(b, nb, add_eng):
                """q/qr conv + rope for one 512-wide t-chunk (feeds attn(b, nb))."""
                s = st[b]
                if nb == 0:
                    s["qrope"] = [
                        mp.tile([128, 4, 512], bf, tag="qrope", bufs=4, name=f"qrope{b}{m}")
                        for m in range(2)
                    ]
                sl = slice(nb * 512, nb * 512 + 512)
                for m in range(2):
                    psq = pp.tile([128, 512], f32, tag="cv", bufs=2, name=f"psq{b}{m}{nb}")
                    psqr = pp.tile([128, 512], f32, tag="cv", bufs=2, name=f"psqr{b}{m}{nb}")
                    nc.tensor.matmul(
                        psq[:], wqT[:, :, m * 128 : m * 128 + 128],
                        s["xf8"][:, :, nb * 512 : nb * 512 + 512],
                        start=True, stop=True, perf_mode=DR,
                    )
                    nc.tensor.matmul(
                        psqr[:], wqrT[:, :, m * 128 : m * 128 + 128],
                        s["xf8"][:, :, nb * 512 : nb * 512 + 512],
                        start=True, stop=True, perf_mode=DR,
                    )
                    t1 = mp.tile([128, 512], bf, tag="qt1", bufs=2, name=f"qt1{b}{m}{nb}")
                    nc.vector.scalar_tensor_tensor(
                        t1[:], psq[:], bq_(m), cosq[:, sl], op0=Alu.add, op1=Alu.mult
                    )
                    t2 = mp.tile([128, 512], bf, tag="qt2", bufs=2, name=f"qt2{b}{m}{nb}")
                    nc.vector.scalar_tensor_tensor(
                        t2[:], psqr[:], bqr_(m), sinq[:, sl], op0=Alu.add, op1=Alu.mult
                    )
                    add_eng.tensor_add(s["qrope"][m][:, nb, :], t1[:], t2[:])

            def attn(b, tq):
                s = st[b]
                if tq == 0:
                    s["ntp"] = {}
                ntp = mp.tile([128, 2, 512], f8, tag="ntp", bufs=3, name=f"ntp{b}{tq}")
                s["ntp"][tq] = ntp
                ps_p = {}
                for hp in range(2):
                    # p tile [s128, head-in-pair 2, sc 4, t 512]
                    p = mp.tile([128, 2, 4, 512], f8, tag="p", bufs=3, name=f"p{b}{tq}{hp}")
                    ps_p[hp] = p
                for h in range(H):
                    hp, hh = divmod(h, 2)
                    chq = h // 2
                    base = 64 * (h % 2)
                    p = ps_p[hp]
                    for e in range(2):
                        pss = pp.tile([128, 2, 512], f32, tag="pss", bufs=2,
                                      name=f"pss{b}{tq}{h}{e}")
                        for j in range(2):
                            sc = 2 * e + j
                            nc.tensor.matmul(
                                pss[:, j, :],
                                s["krope"][chq][base : base + 64, sc * 128 : sc * 128 + 128],
                                s["qrope"][chq][base : base + 64, tq, :],
                                start=True, stop=True,
                            )
                        nc.scalar.activation(
                            p[:, hh, 2 * e : 2 * e + 2, :], pss[:], Act.Exp, scale=0.125
                        )
                    if h % 2 == 1:
                        # pair complete: attention out + Z + normalize
                        p = ps_p[hp]
                        pso = pp.tile([128, 512], f32, tag="att", bufs=2,
                                      name=f"pso{b}{tq}{hp}")
                        for sc in range(4):
                            nc.tensor.matmul(
                                pso[:], s["vt"][:, sc, hp, :, :], p[:, :, sc, :],
                                start=(sc == 0), stop=(sc == 3), perf_mode=DR,
                            )
                        zb = pp.tile([128, 512], f32, tag="att", bufs=2,
                                     name=f"zb{b}{tq}{hp}")
                        for sc in range(4):
                            nc.tensor.matmul(
                                zb[:], oj[:], p[:, :, sc, :],
                                start=(sc == 0), stop=(sc == 3), perf_mode=DR,
                            )
                        zr = mp.tile([128, 512], f32, tag="zr", bufs=2,
                                     name=f"zr{b}{tq}{hp}")
                        nc.vector.reciprocal(zr[:], zb[:])
                        # ntp = pso * zr = (64*a_hat) / (4*Z) = 16*a
                        nc.vector.tensor_mul(ntp[:, hp, :], pso[:], zr[:])

            def film(b, tq, beta_on_act):
                s = st[b]
                if tq == 0:
                    s["ft"] = []
                    for ch in range(2):
                        ft = mp.tile([128, TT], f32, tag="ft", bufs=4, name=f"ft{b}{ch}")
                        s["ft"].append(ft)
                ntp = s["ntp"][tq]
                tsl = slice(tq * 512, tq * 512 + 512)
                for ch in range(2):
                    psf = pp.tile([128, 2, 512], f32, tag="pss", bufs=2,
                                  name=f"psf{b}{tq}{ch}")
                    nc.tensor.matmul(
                        psf[:, 0, :], wfoT[:, :, ch * 128 : ch * 128 + 128], ntp[:],
                        start=True, stop=True, perf_mode=DR,
                    )
                    nc.tensor.matmul(
                        psf[:, 1, :], wfoT[:, :, 256 + ch * 128 : 256 + ch * 128 + 128],
                        ntp[:],
                        start=True, stop=True, perf_mode=DR,
                    )
                    # tg = (psf_gamma/128) * x ; psf = (8wfo)@(16a) = 128*gamma
                    tg = mp.tile([128, 512], f32, tag="tg", bufs=2, name=f"tg{b}{tq}{ch}")
                    nc.vector.scalar_tensor_tensor(
                        tg[:], psf[:, 0, :], 1.0 / 128.0, s["x32"][ch][:, tsl],
                        op0=Alu.mult, op1=Alu.mult,
                    )
                    ftsl = s["ft"][ch][:, tsl]
                    if beta_on_act:
                        # evict beta on the scalar engine, add on Pool (both
                        # off the DVE critical path)
                        fb = mp.tile([128, 512], f32, tag="fb", bufs=2,
                                     name=f"fb{b}{tq}{ch}")
                        if has_bias:
                            nc.scalar.activation(
                                fb[:], psf[:, 1, :], Act.Identity,
                                bias=bfb_(ch), scale=1.0 / 128.0,
                            )
                        else:
                            nc.scalar.activation(
                                fb[:], psf[:, 1, :], Act.Copy, scale=1.0 / 128.0,
                            )
                        nc.gpsimd.tensor_add(ftsl, fb[:], tg[:])
                    else:
                        nc.vector.scalar_tensor_tensor(
                            ftsl, psf[:, 1, :], 1.0 / 128.0, tg[:],
                            op0=Alu.mult, op1=Alu.add,
                        )
                        if has_bias:
                            nc.vector.tensor_scalar_add(ftsl, ftsl, bfb_(ch))
                    if has_bias:
                        # y += x*b2gamma (skipped when biases are zero)
                        nc.vector.scalar_tensor_tensor(
                            ftsl, s["x32"][ch][:, tsl], bfg_(ch), ftsl,
                            op0=Alu.mult, op1=Alu.add,
                        )
                    nc.sync.dma_start(
                        d_out[b, ch * 128 : ch * 128 + 128, tsl], ftsl
                    )

            # ---- emission schedule: per-t-chunk pipeline, overlap batch-1
            # convs with batch-0 attention ----
            # ---- emission schedule.  Each engine executes its instructions
            # in emission order, so ops are placed per-engine to avoid
            # head-of-line blocking on not-yet-ready inputs. ----
            V, G = nc.vector, nc.gpsimd
            # DMA queue: cond-side minimum, then q-side critical path
            load_weights_early()          # + DVE cosk/sink mirrors
            load_tables_q()               # + DVE cosq/sinq mirrors
            load_cond_dma(0)
            load_x_dma(0, 0)
            load_weights_late()
            load_x_dma(0, 1)
            load_wfo()
            cast_cond(0, split=True)      # DVE pair0, Pool pair1
            conv_c(0)
            conv_k(0, add_eng=V)          # DVE stt + DVE adds (prologue-fast)
            cast_x(0, 0, V)
            conv_vt(0)
            conv_q(0, 0, add_eng=V)
            conv_q(0, 1, add_eng=V)
            cast_x(0, 1, G)               # Pool: second x half (needed nb>=2)
            attn(0, 0)
            load_cond_dma(1); load_x_dma(1, 0); load_x_dma(1, 1)
            conv_q(0, 2, add_eng=G); attn(0, 1); film(0, 0, True)
            cast_cond(1, split=False); cast_x(1, 0, G); cast_x(1, 1, G)
            conv_q(0, 3, add_eng=G); attn(0, 2); film(0, 1, True); conv_c(1)
            conv_k(1, add_eng=G); conv_vt(1)
            conv_q(1, 0, add_eng=G); attn(0, 3); film(0, 2, True)
            conv_q(1, 1, add_eng=G); attn(1, 0); film(0, 3, True)
            conv_q(1, 2, add_eng=G); attn(1, 1); film(1, 0, False)
            conv_q(1, 3, add_eng=G); attn(1, 2); film(1, 1, False)
            attn(1, 3); film(1, 2, False)
            film(1, 3, False)

    nc.compile()
    return nc


def _host_prep(inputs):
    wq, bq = inputs["wq"], inputs["bq"]
    wk, bk = inputs["wk"], inputs["bk"]
    wv, bv = inputs["wv"], inputs["bv"]
    wc, bc = inputs["w_cond"], inputs["b_cond"]
    wo = inputs["wo"]
    wf, bf_ = inputs["w_film"], inputs["b_film"]

    cosq, sinq = _rope_tables(TT)
    cosk, sink = _rope_tables(TS)
    # fold output projection and bv/bo into the film conv (host fp64)
    wfo = (wf.astype(np.float64) @ wo.astype(np.float64)).astype(np.float32)
    b2 = (
        wfo.astype(np.float64) @ bv.astype(np.float64)
        + wf.astype(np.float64) @ inputs["bo"].astype(np.float64)
        + bf_
    ).astype(np.float32)

    ojv = np.zeros((128, 2, 128), np.float32)
    ojv[:, 0, 0:64] = 4.0
    ojv[:, 1, 64:128] = 4.0

    bias = np.zeros((128, 14), np.float32)
    bias[:, 0:2] = _colchunks(bc, 2) * WS
    bias[:, 2:4] = _colchunks(bq, 2) * WS
    bias[:, 4:6] = _colchunks(_rot_fold(bq[:, None])[:, 0], 2) * WS
    bias[:, 6:8] = _colchunks(bk, 2) * WS * WS
    bias[:, 8:10] = _colchunks(_rot_fold(bk[:, None])[:, 0], 2) * WS * WS
    bias[:, 10:12] = _colchunks(b2[:HIDDEN], 2)
    bias[:, 12:14] = _colchunks(b2[HIDDEN:], 2)

    shared = {
        "cosq": _b16(cosq / WS), "sinq": _b16(sinq / WS),
        "cosk": _b16(cosk / (WS * WS)), "sink": _b16(sink / (WS * WS)),
        "wcT": _f8(_chunkT(wc, 4) * WS),
        "wqT": _f8(_chunkT(wq, 2) * WS),
        "wqrT": _f8(_chunkT(_rot_fold(wq), 2) * WS),
        "wkT": _f8(_chunkT(wk, 2) * WS),
        "wkrT": _f8(_chunkT(_rot_fold(wk), 2) * WS),
        "wvT": _f8(_chunkT(wv, 2) * WS),
        "wfoT": _f8(_chunkT(wfo, 2) * WS),
        "oj": _f8(ojv),
        "bias": np.ascontiguousarray(bias),
    }
    has_bias = bool(np.any(b2 != 0.0))
    return shared, has_bias


def kernel(**inputs):
    from concourse.bass_utils import run_bass_kernel_spmd

    inputs = {k: np.asarray(v, dtype=np.float32) for k, v in inputs.items()}
    shared, has_bias = _host_prep(inputs)

    key = ("nc", has_bias)
    if key not in _CACHE:
        _CACHE["nc"] = _build_program(has_bias)
        _CACHE[key] = _CACHE["nc"]
    nc = _CACHE[key]

    x = inputs["x"]
    cond = inputs["cond_latent"]
    in_maps = []
    for c in range(N_CORES):
        m = dict(shared)
        m["x"] = np.ascontiguousarray(x[c * BPC : (c + 1) * BPC])
        m["cond"] = np.ascontiguousarray(cond[c * BPC : (c + 1) * BPC])
        in_maps.append(m)

    res = run_bass_kernel_spmd(nc, in_maps, list(range(N_CORES)))
    out = np.concatenate([res.results[c]["out"] for c in range(N_CORES)], axis=0)
    return out.astype(np.float32)
